# revision 1
# baseline (speedup 1.0000x reference)
"""3-layer GAT + global-mean-pool + FC on 8 Trainium2 NeuronCores.

Strategy (graph/data parallel, per the sharding hint):
  - Nodes (and their incident edges, by dst) are sharded across 8 cores.
  - Per layer, each core computes h = x @ W_ext for its node slice (W_ext also
    produces the per-node attention terms al_src/al_dst as extra columns),
    writes the rows to a DRAM table, and an AllGather replicates the full
    table to every core.
  - Edges are dst-sorted into 128-dst windows; messages h[src] are fetched
    with dma_gather (row gather), attention logits use the gathered al_src
    plus a second small gather of the local al_dst table, and the per-window
    segment-sum (and softmax denominator) is one matmul per 128-edge block
    with a compare-built 0/1 edge->dst matrix as the stationary operand.
  - LayerNorm+ELU run batched on the node-major result; the final mean-pool
    is another compare-matmul followed by an AllReduce and a tiny FC.

Weight folding (al = h @ a  ->  x @ (W @ a)), channel interleaving and all
integer edge-structure preprocessing happen on the host; all O(N), O(E)
floating-point work runs on device.
"""

import os
import sys

GAT_LAYERS = int(os.environ.get("GAT_LAYERS", "3"))
GAT_NO_AG = os.environ.get("GAT_NO_AG", "0") == "1"
GAT_NO_GATHER = os.environ.get("GAT_NO_GATHER", "0") == "1"
GAT_NO_MM = os.environ.get("GAT_NO_MM", "0") == "1"
GAT_NO_AR = os.environ.get("GAT_NO_AR", "0") == "1"
GAT_NO_POOL = os.environ.get("GAT_NO_POOL", "0") == "1"
GAT_NO_INV = os.environ.get("GAT_NO_INV", "0") == "1"
GAT_NO_FC = os.environ.get("GAT_NO_FC", "0") == "1"
GAT_DEBUG = os.environ.get("GAT_DEBUG", "0") == "1"


for _p in ("/opt/trn_rl_repo", "/opt/pypackages"):
    if _p not in sys.path:
        sys.path.append(_p)

import numpy as np

import concourse.bass as bass
import concourse.bacc as bacc
import concourse.tile as tile
import concourse.mybir as mybir
from concourse import library_config
from concourse.bass_utils import run_bass_kernel_spmd

# ---- problem constants (hardcoded per spec) ----
N = 50000
E0 = 800000
NCORES = 8
PARTN = N // NCORES          # 6250 real nodes per core
EMBED = 256
HIDDEN = 64
HEADS = 4
NG = 64                      # graphs
EPS = 1e-5
NEG = 0.2
DW = 128                     # dst window
NW = (PARTN + DW - 1) // DW  # 49 windows per core
NT = NW                      # node tiles per core (node = 128*t + p)
PADN = NT * 128              # 6272 padded nodes per core (table rows per rank)
NPAD = NCORES * PADN         # 50176 global padded table rows
SPLIT = 32768                # int16 index split for src gather

F32 = mybir.dt.float32
BF16 = mybir.dt.bfloat16
I16 = mybir.dt.int16

# table dtype config
USE_BF16 = os.environ.get("GAT_BF16", "1") == "1"
if USE_BF16:
    TROW = 384               # bf16 cols per table row (768B, %256)
    TDT = BF16
    CHUNK = int(os.environ.get("GAT_CHUNK", "8"))
else:
    TROW = 320               # f32 cols per table row (1280B, %256)
    TDT = F32
    CHUNK = int(os.environ.get("GAT_CHUNK", "6"))

ALS0 = 256                   # col where al_src lives in the table row (f32)
# consts tile column layout
C_BIAS = [0, 256, 512]
C_G = [768, 1024, 1280]
C_BE = [1536, 1792, 2048]
C_FCB = 2304
C_INV = 2560
C_EPS = 2624
C_R256 = 2625
C_IOTAC = 2626
C_SEL = 2688
CW = 3712
JH = 264                     # h_ext width: 256 h + 4 al_s + 4 al_d


def _interleave_perm():
    """col 4c+h <- 64h+c for H=4, C=64."""
    p = np.zeros(256, np.int64)
    for h in range(HEADS):
        for c in range(HIDDEN):
            p[4 * c + h] = 64 * h + c
    return p


def _prep_weights(ws):
    """Fold attention vectors into W, apply channel interleave permutations.

    Returns per-layer W_ext [256, 264] plus permuted ln/bias vectors.
    """
    perm = _interleave_perm()
    out = {}
    in_perm = np.arange(256)
    for l in range(3):
        W = ws[f"w{l}"].astype(np.float64)
        a_s = ws[f"as{l}"].astype(np.float64)
        a_d = ws[f"ad{l}"].astype(np.float64)
        heads = HEADS if l < 2 else 1
        outc = HIDDEN if l < 2 else EMBED
        # WA[cin, h] = sum_c W[cin, h*outc + c] * a[h, c]
        Wr = W.reshape(256, heads, outc)
        wa_s = np.einsum("khc,hc->kh", Wr, a_s)
        wa_d = np.einsum("khc,hc->kh", Wr, a_d)
        Wc = W.copy()
        out_perm = np.arange(256)
        if l < 2:
            Wc = Wc[:, perm]
            out_perm = perm
        Wx = np.zeros((256, JH), np.float64)
        Wx[:, :256] = Wc
        Wx[:, 256 : 256 + heads] = wa_s
        Wx[:, 260 : 260 + heads] = wa_d
        Wx = Wx[in_perm, :]  # permute input rows to match previous layer's interleave
        out[f"wext{l}"] = Wx.astype(np.float32)
        out[f"bias{l}"] = ws[f"b{l}"][out_perm].astype(np.float32)
        out[f"g{l}"] = ws[f"g{l}"][out_perm].astype(np.float32)
        out[f"be{l}"] = ws[f"be{l}"][out_perm].astype(np.float32)
        in_perm = out_perm
    out["fc_w"] = ws["fc_w"].astype(np.float32)  # layer-2 out is plain
    out["fc_b"] = ws["fc_b"].astype(np.float32)
    return out


def _prep_edges(edge_index, batch):
    """Partition/sort/pad edge structure (vectorized). Returns shared block
    structure and per-core index arrays."""
    src = np.concatenate([edge_index[0].astype(np.int64), np.arange(N)])
    dst = np.concatenate([edge_index[1].astype(np.int64), np.arange(N)])
    # map src node id to padded table row id
    spad = (src // PARTN) * PADN + (src % PARTN)
    core = dst // PARTN

    per_core = []
    nblk = np.zeros((NCORES, NW, 2), np.int64)
    for c in range(NCORES):
        m = core == c
        s, d = spad[m], dst[m] - c * PARTN
        w = d // DW
        half = (s >= SPLIT).astype(np.int64)
        order = np.lexsort((s, half, w))
        s, d, w, half = s[order], d[order], w[order], half[order]
        per_core.append((s, d, w, half))
        cnts = np.bincount(w * 2 + half, minlength=NW * 2).reshape(NW, 2)
        nblk[c] = (cnts + 127) // 128

    NBLH = nblk.max(axis=0)  # [NW, 2]

    # shared block list: per window, lo blocks then hi blocks
    blocks = []  # (window, half, first, last)
    blk_start = np.zeros((NW, 2), np.int64)
    pos = 0
    for wi in range(NW):
        tot = int(NBLH[wi, 0] + NBLH[wi, 1])
        blk_start[wi, 0] = pos
        for b in range(int(NBLH[wi, 0])):
            blocks.append((wi, 0, b == 0, b + 1 == tot))
        pos += int(NBLH[wi, 0])
        blk_start[wi, 1] = pos
        for b in range(int(NBLH[wi, 1])):
            blocks.append(
                (wi, 1, NBLH[wi, 0] == 0 and b == 0, b + 1 + NBLH[wi, 0] == tot)
            )
        pos += int(NBLH[wi, 1])
    NBLK = len(blocks)

    sidx = np.zeros((NCORES, 128, NBLK * 8), np.int16)
    didx = np.zeros((NCORES, 128, NBLK * 8), np.int16)
    dloc = np.full((NCORES, 128, NBLK), -1.0, np.float32)

    for c in range(NCORES):
        s, d, w, half = per_core[c]
        n = len(s)
        g = w * 2 + half  # sorted group key
        starts = np.r_[0, np.flatnonzero(np.diff(g)) + 1]
        gstart = starts[np.searchsorted(g[starts], g)]
        pos_in = np.arange(n) - gstart
        blk = blk_start[w, half] + pos_in // 128
        p = pos_in % 128
        rows = p % 16
        cols = blk * 8 + p // 16
        sc = np.zeros((128, NBLK * 8), np.int16)
        dc = np.zeros((128, NBLK * 8), np.int16)
        sc[rows, cols] = s - half * SPLIT
        dc[rows, cols] = d
        for k in range(1, 8):
            sc[16 * k : 16 * k + 16] = sc[:16]
            dc[16 * k : 16 * k + 16] = dc[:16]
        sidx[c] = sc
        didx[c] = dc
        dl = np.full((128, NBLK), -1.0, np.float32)
        dl[p, blk] = (d - w * DW).astype(np.float32)
        dloc[c] = dl

    # graph ids per node slot, and counts
    b64 = batch.astype(np.int64)
    nl = (np.arange(128)[:, None] + 128 * np.arange(NT)[None, :])  # [128, NT]
    gid = np.zeros((NCORES, 128, NT), np.float32)
    for c in range(NCORES):
        valid = nl < PARTN
        gv = b64[c * PARTN + np.minimum(nl, PARTN - 1)]
        gid[c] = np.where(valid, gv.astype(np.float32), -1.0)
    cnt = np.bincount(b64, minlength=NG).astype(np.float64)
    inv_cnt = (1.0 / np.maximum(cnt, 1.0)).astype(np.float32)

    return blocks, NBLK, sidx, didx, dloc, gid, inv_cnt


_PROGRAM_CACHE = {}
_LAST_RESULT = None


def _build_program(blocks, NBLK):
    nc = bacc.Bacc("TRN2", target_bir_lowering=False, debug=False, num_devices=NCORES)

    # ---- inputs ----
    xT0 = nc.dram_tensor("xT0", [128, 2, PADN], F32, kind="ExternalInput").ap()
    wext = [
        nc.dram_tensor(f"wext{l}", [256, JH], F32, kind="ExternalInput").ap()
        for l in range(3)
    ]
    fcw = nc.dram_tensor("fcw", [256, 256], F32, kind="ExternalInput").ap()
    sidx = nc.dram_tensor("sidx", [128, NBLK * 8], I16, kind="ExternalInput").ap()
    dloc = nc.dram_tensor("dloc", [128, NBLK], F32, kind="ExternalInput").ap()
    gid = nc.dram_tensor("gid", [128, NT], F32, kind="ExternalInput").ap()
    iota = nc.dram_tensor("iota", [128, 128], F32, kind="ExternalInput").ap()
    ident = nc.dram_tensor("ident", [128, 128], F32, kind="ExternalInput").ap()
    # consts: per layer bias/g/be (256 each), fcb 256, invcnt 64, scalars
    consts = nc.dram_tensor("consts", [128, CW], F32, kind="ExternalInput").ap()
    out_t = nc.dram_tensor("out", [NG, 256], F32, kind="ExternalOutput").ap()
    xdumps = (
        [
            nc.dram_tensor(f"xdump{l}", [128, NT, 256], F32, kind="ExternalOutput").ap()
            for l in range(3)
        ]
        if GAT_DEBUG
        else None
    )
    if GAT_DEBUG:
        aldump = nc.dram_tensor("aldump", [PADN, 8], F32, kind="ExternalOutput").ap()
        exdump = nc.dram_tensor("exdump", [128, CHUNK, 4], F32, kind="ExternalOutput").ap()
        nardump = nc.dram_tensor("nardump", [128, CHUNK, 4], F32, kind="ExternalOutput").ap()
        alddump = nc.dram_tensor("alddump", [128, CHUNK, 8], F32, kind="ExternalOutput").ap()
        gdump = nc.dram_tensor("gdump", [128, CHUNK, 16], F32, kind="ExternalOutput").ap()

    mm = mybir.AluOpType

    import contextlib

    with tile.TileContext(nc) as tc, contextlib.ExitStack() as _ctx:
        if True:
            cpool = _ctx.enter_context(tc.tile_pool(name="const", bufs=1))
            spool = _ctx.enter_context(tc.tile_pool(name="struct", bufs=1))
            wpool = _ctx.enter_context(tc.tile_pool(name="w", bufs=1))
            xtpool = _ctx.enter_context(tc.tile_pool(name="xt", bufs=1))
            xspool = _ctx.enter_context(tc.tile_pool(name="xs", bufs=1))
            epool = _ctx.enter_context(tc.tile_pool(name="evac", bufs=2))
            gpool = _ctx.enter_context(tc.tile_pool(name="gath", bufs=2))
            npool = _ctx.enter_context(tc.tile_pool(name="nar", bufs=2))
            scrpool = _ctx.enter_context(tc.tile_pool(name="scr", bufs=2))
            smpool = _ctx.enter_context(tc.tile_pool(name="small", bufs=2))
            phpool = _ctx.enter_context(tc.tile_pool(name="ph", bufs=1, space="PSUM"))
            paggpool = _ctx.enter_context(tc.tile_pool(name="pagg", bufs=2, space="PSUM"))
            ptpool = _ctx.enter_context(tc.tile_pool(name="pt", bufs=2, space="PSUM"))
            pdtpool = _ctx.enter_context(tc.tile_pool(name="pdt", bufs=1, space="PSUM"))
            paldpool = _ctx.enter_context(tc.tile_pool(name="pald", bufs=1, space="PSUM"))
            pfcpool = _ctx.enter_context(tc.tile_pool(name="pfc", bufs=1, space="PSUM"))
            dram = _ctx.enter_context(tc.tile_pool(name="dram", bufs=1, space="DRAM"))
            nc.gpsimd.load_library(library_config.mlp)

            # persistent SBUF loads
            iota_sb = cpool.tile([128, 128], F32, tag="iota")
            ident_sb = cpool.tile([128, 128], F32, tag="ident")
            consts_sb = cpool.tile([128, CW], F32, tag="consts")
            dloc_sb = spool.tile([128, NBLK], F32, tag="dloc")
            gid_sb = spool.tile([128, NT], F32, tag="gid")
            nc.sync.dma_start(iota_sb[:], iota[:])
            nc.sync.dma_start(ident_sb[:], ident[:])
            nc.sync.dma_start(consts_sb[:], consts[:])
            nc.sync.dma_start(dloc_sb[:], dloc[:])
            nc.sync.dma_start(gid_sb[:], gid[:])

            xT = xtpool.tile([128, 2, PADN], F32, tag="xT")
            nc.sync.dma_start(xT[:], xT0[:])

            x_stage = xspool.tile([128, NT, 256], F32, tag="xstage")

            # DRAM tiles
            tab_slice = dram.tile([PADN, TROW], TDT)
            al_slice = dram.tile([PADN, 64], F32)
            ar_in = dram.tile([NG, 256], F32)
            ar_out = dram.tile([NG, 256], F32, addr_space="Shared")

            tab_slice_v = tab_slice[:].rearrange("(t p) r -> p t r", p=128)
            al_slice_v = al_slice[:].rearrange("(t p) r -> p t r", p=128)

            # chunk plan: list of (c0, cb) over blocks
            chunks = []
            c0 = 0
            while c0 < NBLK:
                cb = min(CHUNK, NBLK - c0)
                chunks.append((c0, cb))
                c0 += cb

            def expand_ald(l, c0, cb, ALD, alw_sb):
                # per-edge al_dst via PE expansion instead of a gather:
                # dT = dloc chunk transposed; rep_b = ones x dT[b] (row
                # replicated to 128 partitions); cmpT = (rep == partition);
                # alD[:, b] = cmpT matmul with the window al values.
                dt_ps = pdtpool.tile([CHUNK, 128], F32, tag="dt", name=f"dt{l}_{c0}")
                nc.tensor.transpose(
                    dt_ps[0:cb, :], dloc_sb[:, c0 : c0 + cb], ident_sb[:]
                )
                dt_sb = npool.tile([CHUNK, 128], F32, tag="dtsb", name=f"dts{l}_{c0}")
                nc.vector.tensor_copy(dt_sb[0:cb, :], dt_ps[0:cb, :])
                iotac_b = consts_sb[:, C_IOTAC : C_IOTAC + 1].broadcast_to([128, 128])
                ald_ps = paldpool.tile(
                    [128, 4 * CHUNK], F32, tag="aldps", name=f"alp{l}_{c0}"
                )
                rep_tiles = {}
                for j in range((cb + 3) // 4):
                    rep_tiles[j] = ptpool.tile(
                        [128, 4, 128], F32, tag="pt", name=f"rep{l}_{c0}_{j}"
                    )
                    jb = min(4, cb - 4 * j)
                    for q in range(jb):
                        b = 4 * j + q
                        nc.tensor.matmul(
                            rep_tiles[j][:, q, :],
                            consts_sb[0:cb, C_SEL + 128 * b : C_SEL + 128 * (b + 1)],
                            dt_sb[0:cb, :],
                            start=(q == 0),
                            stop=(q == jb - 1),
                            skip_group_check=True,
                        )
                for b in range(cb):
                    cmpt = npool.tile(
                        [128, 128], F32, tag="cmpt", name=f"ct{l}_{c0}_{b}"
                    )
                    nc.vector.tensor_tensor(
                        cmpt[:], rep_tiles[b // 4][:, b % 4, :], iotac_b, mm.is_equal
                    )
                    wi_b = blocks[c0 + b][0]
                    nc.tensor.matmul(
                        ald_ps[:, 4 * b : 4 * b + 4],
                        cmpt[:],
                        alw_sb[:, wi_b, :],
                        start=(b == 0),
                        stop=(b == cb - 1),
                        skip_group_check=True,
                    )
                nc.vector.tensor_copy(
                    ALD[:, 0:cb, 0:4],
                    ald_ps[:, 0 : 4 * cb].rearrange("p (b a) -> p b a", a=4),
                )

            for l in range(GAT_LAYERS):
                nh = HEADS if l < 2 else 1
                tab_full = dram.tile(
                    [NPAD, TROW], TDT, addr_space="Shared", name=f"tab_full{l}"
                )
                wsb = wpool.tile([128, 2, JH], F32, tag="w")
                nc.sync.dma_start(wsb[:], wext[l].rearrange("(k p) j -> p k j", p=128))

                # ---- phase 1: h_ext slice + table/al writes ----
                for nb in range(NT):
                    ph = phpool.tile([128, JH], F32, tag="ph")
                    for kc in range(2):
                        nc.tensor.matmul(
                            ph[:],
                            xT[:, kc, nb * 128 : (nb + 1) * 128],
                            wsb[:, kc, :],
                            start=(kc == 0),
                            stop=(kc == 1),
                        )
                    ev = epool.tile([128, TROW], TDT, tag="ev")
                    # h channels (cast to table dtype)
                    nc.vector.tensor_copy(ev[:, 0:256], ph[:, 0:256])
                    if USE_BF16:
                        # al_src kept as raw f32 bytes inside the bf16 row
                        als_f32 = ev[:, 256:264].bitcast(F32)
                        nc.vector.tensor_copy(als_f32, ph[:, 256:260])
                    else:
                        nc.vector.tensor_copy(ev[:, 256:260], ph[:, 256:260])
                    alv = epool.tile([128, 8], F32, tag="alv")
                    nc.vector.tensor_copy(alv[:], ph[:, 256:264])
                    nc.sync.dma_start(tab_slice_v[:, nb, :], ev[:])
                    nc.sync.dma_start(al_slice_v[:, nb, 0:8], alv[:])

                alw_sb = wpool.tile([128, NT, 4], F32, tag="alw")
                nc.sync.dma_start(
                    alw_sb[:],
                    al_slice[:, 4:8].rearrange("(t p) r -> p t r", p=128),
                )
                # ---- phase 2: allgather table ----
                if GAT_NO_AG:
                    nc.sync.dma_start(tab_full[0:PADN, :], tab_slice[:])
                else:
                    nc.gpsimd.collective_compute(
                        "AllGather",
                        mm.bypass,
                        replica_groups=[list(range(NCORES))],
                        ins=[tab_slice.opt()],
                        outs=[tab_full.opt()],
                    )
                tab_lo = tab_full[0:SPLIT, :]
                tab_hi = tab_full[SPLIT:NPAD, :]

                # ---- phase 3: gather + aggregate ----
                win_psum = {}
                for c0, cb in chunks:
                    G = gpool.tile([128, CHUNK, TROW], TDT, tag="G")
                    ALD = npool.tile([128, CHUNK, 4], F32, tag="ALD")
                    if GAT_NO_GATHER:
                        nc.vector.memset(G[:], 0.5)
                        nc.vector.memset(ALD[:], 0.25)
                    else:
                        sidx_sb = npool.tile([128, CHUNK * 8], I16, tag="sidxc")
                        nc.sync.dma_start(
                            sidx_sb[:, 0 : cb * 8], sidx[:, c0 * 8 : (c0 + cb) * 8]
                        )
                        # gather runs grouped by src half
                        r0 = 0
                        while r0 < cb:
                            hf = blocks[c0 + r0][1]
                            r1 = r0
                            while r1 < cb and blocks[c0 + r1][1] == hf:
                                r1 += 1
                            nrun = (r1 - r0) * 128
                            nc.gpsimd.dma_gather(
                                G[:, r0:r1, :],
                                tab_lo if hf == 0 else tab_hi,
                                sidx_sb[:, r0 * 8 : r1 * 8],
                                nrun,
                                nrun,
                                TROW,
                            )
                            r0 = r1
                    expand_ald(l, c0, cb, ALD, alw_sb)
                    # narrow attention math
                    nar = npool.tile([128, CHUNK, 4], F32, tag="nar")
                    if USE_BF16:
                        als_ap = G[:, 0:cb, 256:264].bitcast(F32)[:, :, 0:nh]
                    else:
                        als_ap = G[:, 0:cb, 256 : 256 + nh]
                    nc.vector.tensor_tensor(
                        nar[:, 0:cb, 0:nh], als_ap, ALD[:, 0:cb, 0:nh], mm.add
                    )
                    lr = npool.tile([128, CHUNK, 4], F32, tag="lr")
                    nc.vector.tensor_scalar_mul(lr[:, 0:cb, 0:nh], nar[:, 0:cb, 0:nh], NEG)
                    nc.vector.tensor_tensor(
                        nar[:, 0:cb, 0:nh], nar[:, 0:cb, 0:nh], lr[:, 0:cb, 0:nh], mm.max
                    )
                    ex = npool.tile([128, CHUNK, 4], TDT, tag="ex")
                    nc.scalar.activation(
                        ex[:, 0:cb, 0:nh],
                        nar[:, 0:cb, 0:nh],
                        mybir.ActivationFunctionType.Exp,
                    )
                    if GAT_DEBUG and l == 0 and c0 == 0:
                        nc.sync.dma_start(exdump[:, 0:cb, 0:nh], ex[:, 0:cb, 0:nh])
                        nc.sync.dma_start(nardump[:, 0:cb, 0:nh], nar[:, 0:cb, 0:nh])
                        nc.sync.dma_start(alddump[:, :, 0:4], ALD[:, :, 0:4])
                        if USE_BF16:
                            nc.sync.dma_start(
                                gdump[:, 0:cb, 0:4], G[:, 0:cb, 256:264].bitcast(F32)
                            )
                        else:
                            nc.sync.dma_start(gdump[:], G[:, :, 256:272])
                    cmp = npool.tile([128, CHUNK, DW], TDT, tag="cmp")
                    dl_b = dloc_sb[:, c0 : c0 + cb].unsqueeze(2).broadcast_to(
                        [128, cb, DW]
                    )
                    io_b = iota_sb[:].unsqueeze(1).broadcast_to([128, cb, DW])
                    nc.vector.tensor_tensor(cmp[:, 0:cb, :], dl_b, io_b, mm.is_equal)
                    # prescale h channels by ex
                    if l < 2:
                        g_v = G[:, 0:cb, 0:256].rearrange("p b (c h) -> p b c h", h=4)
                        ex_b = ex[:, 0:cb, :].unsqueeze(2).broadcast_to([128, cb, 64, 4])
                        nc.vector.tensor_tensor(g_v, g_v, ex_b, mm.mult)
                    else:
                        ex_b = ex[:, 0:cb, 0:1].broadcast_to([128, cb, 256])
                        nc.vector.tensor_tensor(
                            G[:, 0:cb, 0:256], G[:, 0:cb, 0:256], ex_b, mm.mult
                        )

                    for b in range(cb):
                        if GAT_NO_MM:
                            break
                        wi, hf, first, last = blocks[c0 + b]
                        if first:
                            win_psum[wi] = paggpool.tile([128, 260], F32, tag="pagg", name=f"pagg{wi}")
                        pw = win_psum[wi]
                        # start=True clears the whole bank: issue it exactly
                        # once (den MM of the first block), all else accumulate
                        nc.tensor.matmul(
                            pw[:, 256 : 256 + nh],
                            cmp[:, b, :],
                            ex[:, b, 0:nh],
                            start=first,
                            stop=last,
                            skip_group_check=True,
                        )
                        nc.tensor.matmul(
                            pw[:, 0:256],
                            cmp[:, b, :],
                            G[:, b, 0:256],
                            start=False,
                            stop=last,
                            skip_group_check=True,
                        )
                        if last:
                            den = smpool.tile([128, 4], F32, tag="den")
                            nc.vector.tensor_scalar_max(
                                den[:, 0:nh], pw[:, 256 : 256 + nh], 1e-30
                            )
                            rden = smpool.tile([128, 4], F32, tag="rden")
                            nc.vector.reciprocal(rden[:, 0:nh], den[:, 0:nh])
                            if l < 2:
                                x_v = x_stage[:, wi, :].rearrange(
                                    "p (c h) -> p c h", h=4
                                )
                                p_v = pw[:, 0:256].rearrange("p (c h) -> p c h", h=4)
                                rd_b = rden[:].unsqueeze(1).broadcast_to([128, 64, 4])
                                nc.vector.tensor_tensor(x_v, p_v, rd_b, mm.mult)
                            else:
                                rd_b = rden[:, 0:1].broadcast_to([128, 256])
                                nc.vector.tensor_tensor(
                                    x_stage[:, wi, :], pw[:, 0:256], rd_b, mm.mult
                                )
                            del win_psum[wi]

                if GAT_NO_MM:
                    nc.vector.memset(x_stage[:], 0.125)
                # ---- phase 4: bias + layernorm + elu (batched) ----
                def cvec(col):
                    return (
                        consts_sb[:, col : col + 256]
                        .unsqueeze(1)
                        .broadcast_to([128, NT, 256])
                    )

                xs = x_stage[:]
                nc.vector.tensor_tensor(xs, xs, cvec(C_BIAS[l]), mm.add)
                msum = smpool.tile([128, NT], F32, tag="msum")
                nc.vector.tensor_reduce(msum[:], xs, mybir.AxisListType.X, mm.add)
                mu = smpool.tile([128, NT], F32, tag="mu")
                nc.vector.tensor_scalar_mul(mu[:], msum[:], 1.0 / 256.0)
                mu_b = mu[:].unsqueeze(2).broadcast_to([128, NT, 256])
                nc.vector.tensor_tensor(xs, xs, mu_b, mm.subtract)
                ss = smpool.tile([128, NT], F32, tag="ss")
                for t0 in range(0, NT, 4):
                    t1 = min(t0 + 4, NT)
                    sq = scrpool.tile([128, 4, 256], F32, tag="sq")
                    nc.vector.tensor_tensor(
                        sq[:, 0 : t1 - t0, :],
                        x_stage[:, t0:t1, :],
                        x_stage[:, t0:t1, :],
                        mm.mult,
                    )
                    nc.vector.tensor_reduce(
                        ss[:, t0:t1],
                        sq[:, 0 : t1 - t0, :],
                        mybir.AxisListType.X,
                        mm.add,
                    )
                sd = smpool.tile([128, NT], F32, tag="sd")
                nc.scalar.activation(
                    sd[:],
                    ss[:],
                    mybir.ActivationFunctionType.Sqrt,
                    bias=consts_sb[:, C_EPS : C_EPS + 1],
                    scale=consts_sb[:, C_R256 : C_R256 + 1],
                )
                rstd = smpool.tile([128, NT], F32, tag="rstd")
                nc.vector.reciprocal(rstd[:], sd[:])
                rstd_b = rstd[:].unsqueeze(2).broadcast_to([128, NT, 256])
                nc.vector.tensor_tensor(xs, xs, rstd_b, mm.mult)
                nc.vector.tensor_tensor(xs, xs, cvec(C_G[l]), mm.mult)
                nc.vector.tensor_tensor(xs, xs, cvec(C_BE[l]), mm.add)
                # elu(x) = max(x,0) + min(e^x,1) - 1
                for t0 in range(0, NT, 4):
                    t1 = min(t0 + 4, NT)
                    ee = scrpool.tile([128, 4, 256], F32, tag="sq")
                    nc.scalar.activation(
                        ee[:, 0 : t1 - t0, :],
                        x_stage[:, t0:t1, :],
                        mybir.ActivationFunctionType.Exp,
                    )
                    nc.vector.tensor_scalar(
                        ee[:, 0 : t1 - t0, :],
                        ee[:, 0 : t1 - t0, :],
                        1.0,
                        -1.0,
                        mm.min,
                        mm.add,
                    )
                    nc.vector.tensor_scalar_max(
                        x_stage[:, t0:t1, :], x_stage[:, t0:t1, :], 0.0
                    )
                    nc.vector.tensor_tensor(
                        x_stage[:, t0:t1, :],
                        x_stage[:, t0:t1, :],
                        ee[:, 0 : t1 - t0, :],
                        mm.add,
                    )

                if GAT_DEBUG:
                    nc.sync.dma_start(xdumps[l][:], x_stage[:])
                    if l == 0:
                        nc.sync.dma_start(
                            aldump[:], al_slice[:, 0:8]
                        )
                # ---- phase 5: transpose x for next layer ----
                if l < 2:
                    for nb in range(NT):
                        for kc in range(2):
                            pt = ptpool.tile([128, 128], F32, tag="pt")
                            nc.tensor.transpose(
                                pt[:],
                                x_stage[:, nb, kc * 128 : (kc + 1) * 128],
                                ident_sb[:],
                            )
                            nc.vector.tensor_copy(
                                xT[:, kc, nb * 128 : (nb + 1) * 128], pt[:]
                            )

            if GAT_LAYERS == 0:
                nc.vector.memset(x_stage[:], 0.125)
            # ---- pooling ----
            if GAT_NO_POOL:
                cmpg = None
            cmpg = scrpool.tile([128, NT, NG], F32, tag="cmpg", bufs=1)
            gid_b = gid_sb[:].unsqueeze(2).broadcast_to([128, NT, NG])
            io64_b = iota_sb[:, 0:NG].unsqueeze(1).broadcast_to([128, NT, NG])
            nc.vector.tensor_tensor(cmpg[:], gid_b, io64_b, mm.is_equal)
            pp = pfcpool.tile([NG, 256], F32, tag="pfc")
            if GAT_NO_POOL:
                nc.vector.memset(pp[:], 0.0)
            for b in range(NT) if not GAT_NO_POOL else []:
                nc.tensor.matmul(
                    pp[:],
                    cmpg[:, b, :],
                    x_stage[:, b, :],
                    start=(b == 0),
                    stop=(b == NT - 1),
                )
            pooled = smpool.tile([NG, 256], F32, tag="pooled")
            nc.vector.tensor_copy(pooled[:], pp[:])
            nc.sync.dma_start(ar_in[:], pooled[:])
            if GAT_NO_AR:
                nc.sync.dma_start(ar_out[:], ar_in[:])
            else:
                nc.gpsimd.collective_compute(
                    "AllReduce",
                    mm.add,
                    replica_groups=[list(range(NCORES))],
                    ins=[ar_in.opt()],
                    outs=[ar_out.opt()],
                )
            pooled2 = smpool.tile([NG, 256], F32, tag="pooled2")
            nc.sync.dma_start(pooled2[:], ar_out[:])
            if not GAT_NO_INV:
                nc.vector.tensor_scalar_mul(
                    pooled2[:], pooled2[:], consts_sb[0:NG, C_INV : C_INV + 1]
                )
            # fc
            if GAT_NO_FC:
                nc.sync.dma_start(out_t[:], pooled2[:])
            fcw_sb = wpool.tile([128, 2, 256], F32, tag="fcw")
            nc.sync.dma_start(fcw_sb[:], fcw.rearrange("(k p) j -> p k j", p=128))
            poolT = smpool.tile([128, 2, NG], F32, tag="poolT")
            for kc in range(2):
                pt = ptpool.tile([128, 128], F32, tag="pt")
                nc.tensor.transpose(
                    pt[0:128, 0:NG],
                    pooled2[:, kc * 128 : (kc + 1) * 128],
                    ident_sb[0:NG, 0:NG],
                )
                nc.vector.tensor_copy(poolT[:, kc, :], pt[0:128, 0:NG])
            pfc = pfcpool.tile([NG, 256], F32, tag="pfc")
            for kc in range(2):
                nc.tensor.matmul(
                    pfc[:],
                    poolT[:, kc, :],
                    fcw_sb[:, kc, :],
                    start=(kc == 0),
                    stop=(kc == 1),
                )
            ores = smpool.tile([NG, 256], F32, tag="ores")
            fcb_b = consts_sb[0:NG, C_FCB : C_FCB + 256]
            nc.vector.tensor_tensor(ores[:], pfc[:], fcb_b, mm.add)
            nc.vector.tensor_scalar_max(ores[:], ores[:], 0.0)
            if not GAT_NO_FC:
                nc.sync.dma_start(out_t[:], ores[:])

    nc.compile()
    return nc


def kernel(**inputs):
    x = np.asarray(inputs["x"], np.float32)
    edge_index = np.asarray(inputs["edge_index"])
    batch = np.asarray(inputs["batch"])

    blocks, NBLK, sidx, didx, dloc, gid, inv_cnt = _prep_edges(edge_index, batch)
    wp = _prep_weights(inputs)

    key = (NBLK, USE_BF16, CHUNK, GAT_DEBUG, GAT_LAYERS, GAT_NO_AG, GAT_NO_GATHER, GAT_NO_MM, GAT_NO_AR, GAT_NO_POOL, GAT_NO_INV, GAT_NO_FC, tuple(b for b in blocks[:8]))
    if key not in _PROGRAM_CACHE:
        _PROGRAM_CACHE[key] = _build_program(blocks, NBLK)
    nc = _PROGRAM_CACHE[key]

    iota = np.broadcast_to(np.arange(128, dtype=np.float32), (128, 128)).copy()
    ident = np.eye(128, dtype=np.float32)
    consts = np.zeros((128, CW), np.float32)
    for l in range(3):
        consts[:, C_BIAS[l] : C_BIAS[l] + 256] = wp[f"bias{l}"][None, :]
        consts[:, C_G[l] : C_G[l] + 256] = wp[f"g{l}"][None, :]
        consts[:, C_BE[l] : C_BE[l] + 256] = wp[f"be{l}"][None, :]
    consts[:, C_FCB : C_FCB + 256] = wp["fc_b"][None, :]
    consts[:NG, C_INV] = inv_cnt
    consts[NG:, C_INV] = 1.0
    consts[:, C_EPS] = EPS
    consts[:, C_R256] = 1.0 / 256.0
    consts[:, C_IOTAC] = np.arange(128, dtype=np.float32)
    for q in range(8):
        consts[q, C_SEL + 128 * q : C_SEL + 128 * (q + 1)] = 1.0

    in_maps = []
    for c in range(NCORES):
        xs = np.zeros((PADN, 256), np.float32)
        xs[:PARTN] = x[c * PARTN : (c + 1) * PARTN]
        xT0 = np.ascontiguousarray(
            xs.T.reshape(2, 128, PADN).transpose(1, 0, 2)
        )  # [128, 2, PADN]; xT0[p,k,n] = xs[n, 128k+p]
        in_maps.append(
            {
                "xT0": xT0,
                "wext0": wp["wext0"],
                "wext1": wp["wext1"],
                "wext2": wp["wext2"],
                "fcw": wp["fc_w"],
                "sidx": sidx[c],
                "dloc": dloc[c],
                "gid": gid[c],
                "iota": iota,
                "ident": ident,
                "consts": consts,
            }
        )

    global _LAST_RESULT
    res = run_bass_kernel_spmd(nc, in_maps, core_ids=list(range(NCORES)), trace=False)
    _LAST_RESULT = res
    return res.results[0]["out"]



# revision 8
# speedup vs baseline: 2.3164x; 2.3164x over previous
"""3-layer GAT + global-mean-pool + FC on 8 Trainium2 NeuronCores (v2).

Strategy (graph/data parallel per the sharding hint): nodes and their
incident edges (by dst) are sharded across 8 cores; per layer each core
computes h_ext = x @ W_ext (al_src/al_dst folded in as extra columns), writes
bf16 rows (h+bias | al_src f32 bits) to two half tables, AllGathers them
(split in two so the second half overlaps the first collective), then
aggregates per 128-dst window with one-hot matmuls over dst-sorted
128-edge blocks whose h rows are fetched by pipelined software-DGE gathers
(prepare_only + trigger, so descriptor-gen overlaps DMA).

Key deviations from v1:
  - gathers use prepare_only preps + per-chunk triggers with explicit
    wait_ge on consumer engines -> transfers stream instead of serializing
    with descriptor generation
  - per-edge al_dst comes from a transposed one-hot (cmpT) built via
    gpsimd partition_broadcast of host-laid-out dlocT rows, then a single
    small matmul per block (no f32 expansion matmuls)
  - exp(leaky_relu(z)) computed as max(exp(z), exp(0.2 z)) (exact identity)
    on the scalar engine
  - ex is written next to the gathered h columns so numerator+denominator
    aggregate in ONE matmul per block
  - bias folded into table rows (softmax weights sum to 1); elu+1 stored in
    x_stage with compensations folded into the next layer's weights
  - phase-1 matmuls in float32r, layernorm scale/shift on the scalar engine
"""

import os
import sys

GAT_LAYERS = int(os.environ.get("GAT_LAYERS", "3"))
GAT_NO_AG = os.environ.get("GAT_NO_AG", "0") == "1"
GAT_NO_GATHER = os.environ.get("GAT_NO_GATHER", "0") == "1"
GAT_NO_MM = os.environ.get("GAT_NO_MM", "0") == "1"
GAT_NO_LN = os.environ.get("GAT_NO_LN", "0") == "1"
GAT_NO_TR = os.environ.get("GAT_NO_TR", "0") == "1"
GAT_NO_ALD = os.environ.get("GAT_NO_ALD", "0") == "1"
GAT_NO_PH1 = os.environ.get("GAT_NO_PH1", "0") == "1"

for _p in ("/opt/trn_rl_repo", "/opt/pypackages"):
    if _p not in sys.path:
        sys.path.append(_p)

import numpy as np
import ml_dtypes

import concourse.bass as bass
import concourse.bacc as bacc
import concourse.tile as tile
import concourse.mybir as mybir
from concourse import library_config
from concourse.bass_utils import run_bass_kernel_spmd

# ---- problem constants (hardcoded per spec) ----
N = 50000
NCORES = 8
PARTN = N // NCORES          # 6250 real nodes per core
EMBED = 256
HIDDEN = 64
HEADS = 4
NG = 64                      # graphs
EPS = 1e-5
NEG = 0.2
DW = 128                     # dst window
NW = (PARTN + DW - 1) // DW  # 49 windows per core
NT = NW                      # node tiles per core (node = 128*t + p)
PADN = NT * 128              # 6272 padded nodes per core
NTA = 25                     # tiles in half-table A
NTB = NT - NTA               # 24 tiles in half-table B
ROWA = NTA * 128             # 3200 rows/core in table A
ROWB = NTB * 128             # 3072 rows/core in table B
CHUNK = int(os.environ.get("GAT_CHUNK", "8"))

F32 = mybir.dt.float32
F32R = mybir.dt.float32r
BF16 = mybir.dt.bfloat16
I16 = mybir.dt.int16

TROW = 384                   # bf16 cols per table row (768B, %256)
JH = 264                     # h_ext width: 256 h + 4 al_s + 4 al_d

# consts tile column layout (f32 cols)
C_BIAS = [0, 256, 512]
C_G = [768, 1024, 1280]
C_BE = [1536, 1792, 2048]
C_FCB = 2304
C_INV = 2560
C_EPS = 2624
C_IOTAC = 2625
C_ALSH = [2632, 2640, 2648]  # per-layer [als(4) | ald(4)] shifts
CW = 2688


def _interleave_perm():
    """col 4c+h <- 64h+c for H=4, C=64."""
    p = np.zeros(256, np.int64)
    for h in range(HEADS):
        for c in range(HIDDEN):
            p[4 * c + h] = 64 * h + c
    return p


def _prep_weights(ws):
    """Fold attention vectors into W, apply channel interleave permutations,
    and fold the elu+1 / bias compensations.

    x_stage stores v = elu(y)+1, so layer l>=1 sees x_true = v - 1:
      h_true = v@W - colsum(W)   -> fold -colsum(W) into the table bias
      al_true = v@(Wa) - sum(Wa) -> subtracted at evac via C_ALSH
    The pool/fc tail sees pooled_true = mean(v) - 1 -> fold into fc_b.
    """
    perm = _interleave_perm()
    out = {}
    in_perm = np.arange(256)
    for l in range(3):
        W = ws[f"w{l}"].astype(np.float64)
        a_s = ws[f"as{l}"].astype(np.float64)
        a_d = ws[f"ad{l}"].astype(np.float64)
        heads = HEADS if l < 2 else 1
        outc = HIDDEN if l < 2 else EMBED
        Wr = W.reshape(256, heads, outc)
        wa_s = np.einsum("khc,hc->kh", Wr, a_s)
        wa_d = np.einsum("khc,hc->kh", Wr, a_d)
        Wc = W.copy()
        out_perm = np.arange(256)
        if l < 2:
            Wc = Wc[:, perm]
            out_perm = perm
        Wx = np.zeros((256, JH), np.float64)
        Wx[:, :256] = Wc
        Wx[:, 256 : 256 + heads] = wa_s
        Wx[:, 260 : 260 + heads] = wa_d
        Wx = Wx[in_perm, :]
        out[f"wext{l}"] = Wx.astype(np.float32)
        bias = ws[f"b{l}"].astype(np.float64)[out_perm]
        alsh = np.zeros(8, np.float64)
        if l > 0:
            bias = bias - Wc.sum(axis=0)
            alsh[0:heads] = wa_s.sum(axis=0)
            alsh[4 : 4 + heads] = wa_d.sum(axis=0)
        out[f"bias{l}"] = bias.astype(np.float32)
        out[f"alsh{l}"] = alsh.astype(np.float32)
        out[f"g{l}"] = ws[f"g{l}"][out_perm].astype(np.float32)
        out[f"be{l}"] = ws[f"be{l}"][out_perm].astype(np.float32)
        in_perm = out_perm
    out["fc_w"] = ws["fc_w"].astype(np.float32)
    out["fc_b"] = (
        ws["fc_b"].astype(np.float64) - ws["fc_w"].astype(np.float64).sum(axis=0)
    ).astype(np.float32)
    return out


def _prep_edges(edge_index, batch):
    """Partition/sort/pad edge structure. Blocks are grouped by window PAIR
    then half-table so gather runs are long; per-(window,tab) groups pad to
    128-edge blocks."""
    src = np.concatenate([edge_index[0].astype(np.int64), np.arange(N)])
    dst = np.concatenate([edge_index[1].astype(np.int64), np.arange(N)])
    sloc = src % PARTN
    score = src // PARTN
    tabh = (sloc >= ROWA).astype(np.int64)
    srow = np.where(tabh == 0, score * ROWA + sloc, score * ROWB + (sloc - ROWA))
    core = dst // PARTN

    NPAIR = (NW + 1) // 2
    per_core = []
    nblk = np.zeros((NCORES, NW, 2), np.int64)
    for c in range(NCORES):
        m = core == c
        s, d = srow[m], dst[m] - c * PARTN
        w = d // DW
        th = tabh[m]
        order = np.lexsort((s, w, th, w // 2))
        s, d, w, th = s[order], d[order], w[order], th[order]
        per_core.append((s, d, w, th))
        cnts = np.bincount(w * 2 + th, minlength=NW * 2).reshape(NW, 2)
        nblk[c] = (cnts + 127) // 128

    NBLH = nblk.max(axis=0)  # [NW, 2]

    # shared block list: per pair, per tab, per window
    blocks = []  # (window, tab)
    blk_start = np.zeros((NW, 2), np.int64)
    for pr in range(NPAIR):
        ws_ = [w for w in (2 * pr, 2 * pr + 1) if w < NW]
        for tb in range(2):
            for wi in ws_:
                blk_start[wi, tb] = len(blocks)
                for _ in range(int(NBLH[wi, tb])):
                    blocks.append((wi, tb))
    NBLK = len(blocks)
    # first/last block index per window (PSUM lifecycle)
    wfirst = {}
    wlast = {}
    for bi, (wi, tb) in enumerate(blocks):
        if wi not in wfirst:
            wfirst[wi] = bi
        wlast[wi] = bi

    sidx = np.zeros((NCORES, 128, NBLK * 8), np.int16)
    dloc = np.full((NCORES, 128, NBLK), -1.0, np.float32)
    dlocT = np.full((NCORES, NBLK, 128), -1.0, np.float32)

    for c in range(NCORES):
        s, d, w, th = per_core[c]
        n = len(s)
        g = w * 2 + th  # group key, sorted in block order per pair
        starts = np.r_[0, np.flatnonzero(np.diff(g)) + 1]
        gstart = starts[np.searchsorted(g[starts], g)]
        pos_in = np.arange(n) - gstart
        blk = blk_start[w, th] + pos_in // 128
        p = pos_in % 128
        rows = p % 16
        cols = blk * 8 + p // 16
        sc = np.zeros((128, NBLK * 8), np.int16)
        sc[rows, cols] = s
        for k in range(1, 8):
            sc[16 * k : 16 * k + 16] = sc[:16]
        sidx[c] = sc
        dl = np.full((128, NBLK), -1.0, np.float32)
        dl[p, blk] = (d - w * DW).astype(np.float32)
        dloc[c] = dl
        dlocT[c] = dl.T

    b64 = batch.astype(np.int64)
    nl = np.arange(128)[:, None] + 128 * np.arange(NT)[None, :]
    gid = np.zeros((NCORES, 128, NT), np.float32)
    for c in range(NCORES):
        valid = nl < PARTN
        gv = b64[c * PARTN + np.minimum(nl, PARTN - 1)]
        gid[c] = np.where(valid, gv.astype(np.float32), -1.0)
    cnt = np.bincount(b64, minlength=NG).astype(np.float64)
    inv_cnt = (1.0 / np.maximum(cnt, 1.0)).astype(np.float32)

    return blocks, NBLK, wfirst, wlast, sidx, dloc, dlocT, gid, inv_cnt


_PROGRAM_CACHE = {}
_LAST_RESULT = None


def _build_program(blocks, NBLK, wfirst, wlast):
    nc = bacc.Bacc("TRN2", target_bir_lowering=False, debug=False, num_devices=NCORES)

    xT0 = nc.dram_tensor("xT0", [128, 2, PADN], BF16, kind="ExternalInput").ap()
    wext = [
        nc.dram_tensor(f"wext{l}", [256, JH], BF16, kind="ExternalInput").ap()
        for l in range(3)
    ]
    fcw = nc.dram_tensor("fcw", [256, 256], F32, kind="ExternalInput").ap()
    sidx = nc.dram_tensor("sidx", [128, NBLK * 8], I16, kind="ExternalInput").ap()
    dloc = nc.dram_tensor("dloc", [128, NBLK], BF16, kind="ExternalInput").ap()
    dlocT = nc.dram_tensor("dlocT", [NBLK, 128], BF16, kind="ExternalInput").ap()
    gid = nc.dram_tensor("gid", [128, NT], F32, kind="ExternalInput").ap()
    iota = nc.dram_tensor("iota", [128, 128], F32, kind="ExternalInput").ap()
    ident = nc.dram_tensor("ident", [128, 128], F32, kind="ExternalInput").ap()
    consts = nc.dram_tensor("consts", [128, CW], F32, kind="ExternalInput").ap()
    out_t = nc.dram_tensor("out", [NG, 256], F32, kind="ExternalOutput").ap()

    mm = mybir.AluOpType
    AF = mybir.ActivationFunctionType

    import contextlib

    with tile.TileContext(nc) as tc, contextlib.ExitStack() as _ctx:
        cpool = _ctx.enter_context(tc.tile_pool(name="const", bufs=1))
        spool = _ctx.enter_context(tc.tile_pool(name="struct", bufs=1))
        wpool = _ctx.enter_context(tc.tile_pool(name="w", bufs=1))
        xtpool = _ctx.enter_context(tc.tile_pool(name="xt", bufs=1))
        xspool = _ctx.enter_context(tc.tile_pool(name="xs", bufs=1))
        epool = _ctx.enter_context(tc.tile_pool(name="evac", bufs=2))
        gpool = _ctx.enter_context(tc.tile_pool(name="gath", bufs=3))
        rpool = _ctx.enter_context(tc.tile_pool(name="rep", bufs=2))
        npool = _ctx.enter_context(tc.tile_pool(name="nar", bufs=2))
        scrpool = _ctx.enter_context(tc.tile_pool(name="scr", bufs=2))
        smpool = _ctx.enter_context(tc.tile_pool(name="small", bufs=2))
        phpool = _ctx.enter_context(tc.tile_pool(name="ph", bufs=2, space="PSUM"))
        paggpool = _ctx.enter_context(tc.tile_pool(name="pagg", bufs=2, space="PSUM"))
        paldpool = _ctx.enter_context(tc.tile_pool(name="pald", bufs=2, space="PSUM"))
        ptpool = _ctx.enter_context(tc.tile_pool(name="pt", bufs=1, space="PSUM"))
        pfcpool = _ctx.enter_context(tc.tile_pool(name="pfc", bufs=1, space="PSUM"))
        dram = _ctx.enter_context(tc.tile_pool(name="dram", bufs=1, space="DRAM"))
        nc.gpsimd.load_library(library_config.mlp)

        # persistent SBUF loads
        iota_sb = cpool.tile([128, 128], F32, tag="iota")
        ident_sb = cpool.tile([128, 128], F32, tag="ident")
        consts_sb = cpool.tile([128, CW], F32, tag="consts")
        dloc_sb = spool.tile([128, NBLK], BF16, tag="dloc")
        sidx_sb = spool.tile([128, NBLK * 8], I16, tag="sidx")
        gid_sb = spool.tile([128, NT], F32, tag="gid")
        nc.sync.dma_start(iota_sb[:], iota[:])
        nc.sync.dma_start(ident_sb[:], ident[:])
        nc.sync.dma_start(consts_sb[:], consts[:])
        nc.sync.dma_start(dloc_sb[:], dloc[:])
        nc.sync.dma_start(sidx_sb[:], sidx[:])
        nc.sync.dma_start(gid_sb[:], gid[:])

        xT = xtpool.tile([128, 2, PADN], BF16, tag="xT")
        nc.sync.dma_start(xT[:], xT0[:])
        x_stage = xspool.tile([128, NT, 256], F32, tag="xstage")
        alwb_sb = wpool.tile([128, NT, 4], BF16, tag="alwb")

        gsem = nc.alloc_semaphore("gat_dma")
        nprep = 0  # cumulative preps; each adds 16 to gsem

        # chunk plan
        chunks = []
        c0 = 0
        while c0 < NBLK:
            cb = min(CHUNK, NBLK - c0)
            chunks.append((c0, cb))
            c0 += cb

        iotacb = cpool.tile([128, 1], BF16, tag="iotacb")
        nc.vector.tensor_copy(iotacb[:], consts_sb[:, C_IOTAC : C_IOTAC + 1])
        iotac_col = iotacb[:, 0:1]

        for l in range(GAT_LAYERS):
            nh = HEADS if l < 2 else 1
            tabA_s = dram.tile([ROWA, TROW], BF16, name=f"tabA_s{l}")
            tabB_s = dram.tile([ROWB, TROW], BF16, name=f"tabB_s{l}")
            tabA = dram.tile(
                [NCORES * ROWA, TROW], BF16, addr_space="Shared", name=f"tabA{l}"
            )
            tabB = dram.tile(
                [NCORES * ROWB, TROW], BF16, addr_space="Shared", name=f"tabB{l}"
            )
            tabA_v = tabA_s[:].rearrange("(t p) r -> p t r", p=128)
            tabB_v = tabB_s[:].rearrange("(t p) r -> p t r", p=128)

            wsb = wpool.tile([128, 2, JH], BF16, tag="w")
            nc.sync.dma_start(wsb[:], wext[l].rearrange("(k p) j -> p k j", p=128))

            # ---- phase 1: h_ext + table writes, split A/B for AG overlap ----
            def phase1(t0, t1, tab_v, toff):
                if GAT_NO_PH1:
                    z = epool.tile([128, TROW], BF16, tag="ev")
                    nc.vector.memset(z[:], 0.25)
                    nc.vector.memset(alwb_sb[:], 0.25)
                    for nb in range(t0, t1):
                        nc.sync.dma_start(tab_v[:, nb - toff, :], z[:])
                    return
                for nb in range(t0, t1):
                    ph = phpool.tile([128, JH], F32, tag="ph")
                    for kc in range(2):
                        nc.tensor.matmul(
                            ph[:],
                            xT[:, kc, nb * 128 : (nb + 1) * 128],
                            wsb[:, kc, :],
                            start=(kc == 0),
                            stop=(kc == 1),
                        )
                    ev = epool.tile([128, TROW], BF16, tag="ev")
                    bias_b = consts_sb[:, C_BIAS[l] : C_BIAS[l] + 256]
                    nc.vector.tensor_tensor(ev[:, 0:256], ph[:, 0:256], bias_b, mm.add)
                    # al_src as raw f32 bits; subtract elu+1 compensation
                    als_f32 = ev[:, 256:264].bitcast(F32)
                    nc.vector.tensor_tensor(
                        als_f32,
                        ph[:, 256:260],
                        consts_sb[:, C_ALSH[l] : C_ALSH[l] + 4],
                        mm.subtract,
                    )
                    nc.vector.tensor_tensor(
                        alwb_sb[:, nb, :],
                        ph[:, 260:264],
                        consts_sb[:, C_ALSH[l] + 4 : C_ALSH[l] + 8],
                        mm.subtract,
                    )
                    nc.sync.dma_start(tab_v[:, nb - toff, :], ev[:])

            phase1(0, NTA, tabA_v, 0)
            if GAT_NO_AG:
                nc.sync.dma_start(tabA[0:ROWA, :], tabA_s[:])
            else:
                nc.gpsimd.collective_compute(
                    "AllGather",
                    mm.bypass,
                    replica_groups=[list(range(NCORES))],
                    ins=[tabA_s.opt()],
                    outs=[tabA.opt()],
                )
            phase1(NTA, NT, tabB_v, NTA)
            if GAT_NO_AG:
                nc.sync.dma_start(tabB[0:ROWB, :], tabB_s[:])
            else:
                nc.gpsimd.collective_compute(
                    "AllGather",
                    mm.bypass,
                    replica_groups=[list(range(NCORES))],
                    ins=[tabB_s.opt()],
                    outs=[tabB.opt()],
                )

            # ---- phase 2: gather + aggregate ----
            win_psum = {}
            for c0, cb in chunks:
                G = gpool.tile([128, CHUNK, TROW], BF16, tag="G")
                if GAT_NO_GATHER:
                    nc.vector.memset(G[:], 0.5)
                else:
                    r0 = 0
                    while r0 < cb:
                        tb = blocks[c0 + r0][1]
                        r1 = r0
                        while r1 < cb and blocks[c0 + r1][1] == tb:
                            r1 += 1
                        nrun = (r1 - r0) * 128
                        nc.gpsimd.dma_gather(
                            G[:, r0:r1, :],
                            tabA[:] if tb == 0 else tabB[:],
                            sidx_sb[:, (c0 + r0) * 8 : (c0 + r1) * 8],
                            nrun,
                            nrun,
                            TROW,
                            prepare_only=True,
                            sem=gsem,
                        )
                        nprep += 1
                        r0 = r1
                    nc.gpsimd.trigger_dma(count=None)
                sem_target = 16 * nprep

                # dlocT chunk -> partition 0, then broadcast per block
                dstage = npool.tile([1, CHUNK * 128], BF16, tag="dstage")
                nc.sync.dma_start(
                    dstage[0:1, 0 : cb * 128],
                    dlocT[c0 : c0 + cb, :].rearrange("b e -> (b e)").unsqueeze(0),
                )
                rep = rpool.tile([128, CHUNK, 128], BF16, tag="rep")
                if not GAT_NO_ALD:
                    for b in range(cb):
                        nc.gpsimd.partition_broadcast(
                            rep[:, b, :], dstage[0:1, b * 128 : (b + 1) * 128]
                        )
                iob3 = iotac_col.unsqueeze(1).broadcast_to([128, cb, 128])
                cmpt = rpool.tile([128, CHUNK, 128], BF16, tag="cmpt")
                if GAT_NO_ALD:
                    nc.vector.memset(cmpt[:], 0.0)
                else:
                    nc.vector.tensor_tensor(
                        cmpt[:, 0:cb, :], rep[:, 0:cb, :], iob3, mm.is_equal
                    )
                cmp = rpool.tile([128, CHUNK, DW], BF16, tag="cmp")
                dl_b = dloc_sb[:, c0 : c0 + cb].unsqueeze(2).broadcast_to(
                    [128, cb, DW]
                )
                nc.vector.tensor_tensor(cmp[:, 0:cb, :], dl_b, iob3, mm.is_equal)

                # per-edge al_dst via one small matmul per block
                ald_ps = paldpool.tile([128, 4 * CHUNK], F32, tag="aldps")
                for b in range(cb):
                    wi_b = blocks[c0 + b][0]
                    nc.tensor.matmul(
                        ald_ps[:, 4 * b : 4 * b + 4],
                        cmpt[:, b, :],
                        alwb_sb[:, wi_b, :],
                        start=(b == 0),
                        stop=(b == cb - 1),
                        skip_group_check=True,
                    )

                # attention: ex = max(exp(z), exp(0.2 z)), z = als + ald
                if not GAT_NO_GATHER:
                    nc.vector.wait_ge(gsem, sem_target)
                nar = npool.tile([128, CHUNK, 4], F32, tag="nar")
                als_ap = G[:, 0:cb, 256:264].bitcast(F32)[:, :, 0:nh]
                ald_v = ald_ps[:, 0 : 4 * cb].rearrange("p (b a) -> p b a", a=4)
                nc.vector.tensor_tensor(
                    nar[:, 0:cb, 0:nh], als_ap, ald_v[:, :, 0:nh], mm.add
                )
                e1 = npool.tile([128, CHUNK, 4], F32, tag="e1")
                nc.scalar.activation(
                    e1[:, 0:cb, 0:nh], nar[:, 0:cb, 0:nh], AF.Lrelu, alpha=NEG
                )
                ex_ap = G[:, 0:cb, 256 : 256 + nh]
                nc.scalar.activation(ex_ap, e1[:, 0:cb, 0:nh], AF.Exp)
                # prescale h by ex
                if l < 2:
                    g_v = G[:, 0:cb, 0:256].rearrange("p b (c h) -> p b c h", h=4)
                    ex_b = (
                        G[:, 0:cb, 256:260].unsqueeze(2).broadcast_to([128, cb, 64, 4])
                    )
                    nc.vector.tensor_tensor(g_v, g_v, ex_b, mm.mult)
                else:
                    ex_b = G[:, 0:cb, 256:257].broadcast_to([128, cb, 256])
                    nc.vector.tensor_tensor(
                        G[:, 0:cb, 0:256], G[:, 0:cb, 0:256], ex_b, mm.mult
                    )

                if GAT_NO_MM:
                    continue
                if not GAT_NO_GATHER:
                    nc.tensor.wait_ge(gsem, sem_target)
                for b in range(cb):
                    bi = c0 + b
                    wi = blocks[bi][0]
                    if wfirst[wi] == bi:
                        win_psum[wi] = paggpool.tile(
                            [128, 260], F32, tag="pagg", name=f"pagg{wi}"
                        )
                    pw = win_psum[wi]
                    nc.tensor.matmul(
                        pw[:, 0 : 256 + nh],
                        cmp[:, b, :],
                        G[:, b, 0 : 256 + nh],
                        start=(wfirst[wi] == bi),
                        stop=(wlast[wi] == bi),
                        skip_group_check=True,
                    )
                    if wlast[wi] == bi:
                        den = smpool.tile([128, 4], F32, tag="den")
                        nc.vector.tensor_scalar_max(
                            den[:, 0:nh], pw[:, 256 : 256 + nh], 1e-30
                        )
                        rden = smpool.tile([128, 4], F32, tag="rden")
                        nc.vector.reciprocal(rden[:, 0:nh], den[:, 0:nh])
                        if l < 2:
                            x_v = x_stage[:, wi, :].rearrange("p (c h) -> p c h", h=4)
                            p_v = pw[:, 0:256].rearrange("p (c h) -> p c h", h=4)
                            rd_b = rden[:].unsqueeze(1).broadcast_to([128, 64, 4])
                            nc.vector.tensor_tensor(x_v, p_v, rd_b, mm.mult)
                        else:
                            rd_b = rden[:, 0:1].broadcast_to([128, 256])
                            nc.vector.tensor_tensor(
                                x_stage[:, wi, :], pw[:, 0:256], rd_b, mm.mult
                            )
                        del win_psum[wi]

            if GAT_NO_MM:
                nc.vector.memset(x_stage[:], 0.125)

            # ---- phase 3: layernorm + elu+1 ----
            if GAT_NO_LN:
                if l < 2 and not GAT_NO_TR:
                    for nb in range(NT):
                        for kc in range(2):
                            pt = ptpool.tile([128, 128], F32, tag="pt")
                            nc.tensor.transpose(
                                pt[:],
                                x_stage[:, nb, kc * 128 : (kc + 1) * 128],
                                ident_sb[:],
                            )
                            nc.vector.tensor_copy(
                                xT[:, kc, nb * 128 : (nb + 1) * 128], pt[:]
                            )
                continue
            xs = x_stage[:]
            msum = smpool.tile([128, NT], F32, tag="msum")
            nc.vector.tensor_reduce(msum[:], xs, mybir.AxisListType.X, mm.add)
            mu = smpool.tile([128, NT], F32, tag="mu")
            nc.vector.tensor_scalar_mul(mu[:], msum[:], 1.0 / 256.0)
            ss = smpool.tile([128, NT], F32, tag="ss")
            sqscr = scrpool.tile([128, 256], F32, tag="sqscr")
            for t in range(NT):
                nc.vector.tensor_tensor_reduce(
                    sqscr[:],
                    x_stage[:, t, :],
                    x_stage[:, t, :],
                    1.0,
                    0.0,
                    mm.mult,
                    mm.add,
                    ss[:, t : t + 1],
                )
            q = smpool.tile([128, NT], F32, tag="q")
            nc.vector.tensor_scalar_mul(q[:], ss[:], 1.0 / 256.0)
            m2 = smpool.tile([128, NT], F32, tag="m2")
            nc.vector.tensor_tensor(m2[:], mu[:], mu[:], mm.mult)
            var = smpool.tile([128, NT], F32, tag="var")
            nc.vector.tensor_tensor(var[:], q[:], m2[:], mm.subtract)
            sd = smpool.tile([128, NT], F32, tag="sd")
            nc.scalar.activation(
                sd[:], var[:], AF.Sqrt, bias=consts_sb[:, C_EPS : C_EPS + 1]
            )
            rstd = smpool.tile([128, NT], F32, tag="rstd")
            nc.vector.reciprocal(rstd[:], sd[:])
            nmu = smpool.tile([128, NT], F32, tag="nmu")
            nc.vector.tensor_tensor(nmu[:], mu[:], rstd[:], mm.mult)
            nc.vector.tensor_scalar_mul(nmu[:], nmu[:], -1.0)
            # per-tile (x - mu) * rstd on the scalar engine
            for t in range(NT):
                nc.scalar.activation(
                    x_stage[:, t, :],
                    x_stage[:, t, :],
                    AF.Identity,
                    bias=nmu[:, t : t + 1],
                    scale=rstd[:, t : t + 1],
                )

            def cvec(col):
                return (
                    consts_sb[:, col : col + 256]
                    .unsqueeze(1)
                    .broadcast_to([128, NT, 256])
                )

            nc.vector.tensor_tensor(xs, xs, cvec(C_G[l]), mm.mult)
            nc.vector.tensor_tensor(xs, xs, cvec(C_BE[l]), mm.add)
            # v = elu(y)+1 = max(y,0) + min(e^y, 1)
            for t0 in range(0, NT, 4):
                t1 = min(t0 + 4, NT)
                ee = scrpool.tile([128, 4, 256], F32, tag="ee")
                nc.scalar.activation(
                    ee[:, 0 : t1 - t0, :], x_stage[:, t0:t1, :], AF.Exp
                )
                nc.vector.tensor_scalar_min(ee[:, 0 : t1 - t0, :], ee[:, 0 : t1 - t0, :], 1.0)
                nc.vector.tensor_scalar_max(
                    x_stage[:, t0:t1, :], x_stage[:, t0:t1, :], 0.0
                )
                nc.vector.tensor_tensor(
                    x_stage[:, t0:t1, :],
                    x_stage[:, t0:t1, :],
                    ee[:, 0 : t1 - t0, :],
                    mm.add,
                )

            # ---- phase 4: transpose x for next layer ----
            if l < 2 and not GAT_NO_TR:
                for nb in range(NT):
                    for kc in range(2):
                        pt = ptpool.tile([128, 128], F32, tag="pt")
                        nc.tensor.transpose(
                            pt[:],
                            x_stage[:, nb, kc * 128 : (kc + 1) * 128],
                            ident_sb[:],
                        )
                        nc.vector.tensor_copy(
                            xT[:, kc, nb * 128 : (nb + 1) * 128], pt[:]
                        )

        if GAT_LAYERS == 0:
            nc.vector.memset(x_stage[:], 0.125)
        # ---- pooling (mean(v); the -1 is folded into fc_b) ----
        pp = pfcpool.tile([NG, 256], F32, tag="pfc")
        for b in range(NT):
            cg = smpool.tile([128, NG], F32, tag="cg")
            gid_b = gid_sb[:, b : b + 1].broadcast_to([128, NG])
            nc.vector.tensor_tensor(cg[:], gid_b, iota_sb[:, 0:NG], mm.is_equal)
            nc.tensor.matmul(
                pp[:],
                cg[:],
                x_stage[:, b, :],
                start=(b == 0),
                stop=(b == NT - 1),
            )
        pooled = smpool.tile([NG, 256], F32, tag="pooled")
        nc.vector.tensor_copy(pooled[:], pp[:])
        ar_in = dram.tile([NG, 256], F32)
        ar_out = dram.tile([NG, 256], F32, addr_space="Shared")
        nc.sync.dma_start(ar_in[:], pooled[:])
        nc.gpsimd.collective_compute(
            "AllReduce",
            mm.add,
            replica_groups=[list(range(NCORES))],
            ins=[ar_in.opt()],
            outs=[ar_out.opt()],
        )
        pooled2 = smpool.tile([NG, 256], F32, tag="pooled2")
        nc.sync.dma_start(pooled2[:], ar_out[:])
        nc.vector.tensor_scalar_mul(
            pooled2[:], pooled2[:], consts_sb[0:NG, C_INV : C_INV + 1]
        )
        fcw_sb = wpool.tile([128, 2, 256], F32, tag="fcw")
        nc.sync.dma_start(fcw_sb[:], fcw.rearrange("(k p) j -> p k j", p=128))
        poolT = smpool.tile([128, 2, NG], F32, tag="poolT")
        for kc in range(2):
            pt = ptpool.tile([128, 128], F32, tag="pt")
            nc.tensor.transpose(
                pt[0:128, 0:NG],
                pooled2[:, kc * 128 : (kc + 1) * 128],
                ident_sb[0:NG, 0:NG],
            )
            nc.vector.tensor_copy(poolT[:, kc, :], pt[0:128, 0:NG])
        pfc = pfcpool.tile([NG, 256], F32, tag="pfc")
        for kc in range(2):
            nc.tensor.matmul(
                pfc[:],
                poolT[:, kc, :],
                fcw_sb[:, kc, :],
                start=(kc == 0),
                stop=(kc == 1),
            )
        ores = smpool.tile([NG, 256], F32, tag="ores")
        fcb_b = consts_sb[0:NG, C_FCB : C_FCB + 256]
        nc.vector.tensor_tensor(ores[:], pfc[:], fcb_b, mm.add)
        nc.vector.tensor_scalar_max(ores[:], ores[:], 0.0)
        nc.sync.dma_start(out_t[:], ores[:])

    nc.compile()
    return nc


def kernel(**inputs):
    x = np.asarray(inputs["x"], np.float32)
    edge_index = np.asarray(inputs["edge_index"])
    batch = np.asarray(inputs["batch"])

    blocks, NBLK, wfirst, wlast, sidx, dloc, dlocT, gid, inv_cnt = _prep_edges(
        edge_index, batch
    )
    wp = _prep_weights(inputs)

    key = (NBLK, CHUNK, GAT_LAYERS, GAT_NO_AG, GAT_NO_GATHER, GAT_NO_MM,
           GAT_NO_LN, GAT_NO_TR, GAT_NO_ALD, GAT_NO_PH1,
           tuple(blocks[:8]))
    if key not in _PROGRAM_CACHE:
        _PROGRAM_CACHE[key] = _build_program(blocks, NBLK, wfirst, wlast)
    nc = _PROGRAM_CACHE[key]

    iota = np.broadcast_to(np.arange(128, dtype=np.float32), (128, 128)).copy()
    ident = np.eye(128, dtype=np.float32)
    consts = np.zeros((128, CW), np.float32)
    for l in range(3):
        consts[:, C_BIAS[l] : C_BIAS[l] + 256] = wp[f"bias{l}"][None, :]
        consts[:, C_G[l] : C_G[l] + 256] = wp[f"g{l}"][None, :]
        consts[:, C_BE[l] : C_BE[l] + 256] = wp[f"be{l}"][None, :]
        consts[:, C_ALSH[l] : C_ALSH[l] + 8] = wp[f"alsh{l}"][None, :]
    consts[:, C_FCB : C_FCB + 256] = wp["fc_b"][None, :]
    consts[:NG, C_INV] = inv_cnt
    consts[NG:, C_INV] = 1.0
    consts[:, C_EPS] = EPS
    consts[:, C_IOTAC] = np.arange(128, dtype=np.float32)

    in_maps = []
    for c in range(NCORES):
        xs = np.zeros((PADN, 256), np.float32)
        xs[:PARTN] = x[c * PARTN : (c + 1) * PARTN]
        xT0 = np.ascontiguousarray(
            xs.T.reshape(2, 128, PADN).transpose(1, 0, 2)
        ).astype(ml_dtypes.bfloat16)
        in_maps.append(
            {
                "xT0": xT0,
                "wext0": wp["wext0"].astype(ml_dtypes.bfloat16),
                "wext1": wp["wext1"].astype(ml_dtypes.bfloat16),
                "wext2": wp["wext2"].astype(ml_dtypes.bfloat16),
                "fcw": wp["fc_w"],
                "sidx": sidx[c],
                "dloc": dloc[c].astype(ml_dtypes.bfloat16),
                "dlocT": dlocT[c].astype(ml_dtypes.bfloat16),
                "gid": gid[c],
                "iota": iota,
                "ident": ident,
                "consts": consts,
            }
        )

    global _LAST_RESULT
    res = run_bass_kernel_spmd(nc, in_maps, core_ids=list(range(NCORES)), trace=False)
    _LAST_RESULT = res
    return res.results[0]["out"]
